# revision 25
# baseline (speedup 1.0000x reference)
"""Multi-head self-attention Trainium2 kernel (8-core SPMD), v2.

Problem: B=4, S=2048, E=1024, 16 heads x 64 dim, int mask, softmax attention.

Sharding: core c handles batch b=c//2 and head-half hh=c%2 (8 heads).
Each core computes Yp = Attn(X[b])[:, heads(hh)] @ wO[rows(hh)]  -> [S, E]
partial product (bf16); host sums the two partials per batch and adds bO.

v2 design notes (per core, engine budgets):
  PE ~273us: proj 197k cyc + scores 131k (row-tiled: the two heads of a pair
  use PE row groups 0-63 / 64-127 concurrently) + PV 262k (ones-column rowsum)
  + Y 66k.  ACT ~266us: softmax exp only (256 x [128,1024] tiles), plus QK/
  projection evacuations in the phase-1 window.  DVE: mask multiplies (bf16
  2x, fused over 2 k-tiles), PSUM evacuations, reciprocals.  Pool/GPSIMD:
  partition_broadcast of the reciprocal rowsums (replaces the baseline's DRAM
  round trip).

Loop structure: qc-major (4 query chunks of 512) x p (4 head pairs) x k (16
key tiles).  Projections are software-pipelined into the attention k-loops:
V s-tiles stream just-in-time inside (qc0,p0); K units for pair p inside
(qc0,p); Q units for chunk qc+1 inside (qc,p); Y(qc) is emitted after each
qc so the output projection overlaps later chunks' attention.

PSUM (8 banks): s_pair [128,1024] x2 bufs (4) + proj/Y [128,512] x2 (2) +
PV accumulator [65,1024] x1 (2).
"""

import sys

if "/opt/trn_rl_repo" not in sys.path:
    sys.path.insert(0, "/opt/trn_rl_repo")

import numpy as np
import ml_dtypes

import concourse.bass as bass
import concourse.tile as tile
from concourse import bacc, mybir
from concourse.bass_utils import run_bass_kernel_spmd

F32 = mybir.dt.float32
BF16 = mybir.dt.bfloat16
AF = mybir.ActivationFunctionType

S = 2048      # sequence length
E = 1024      # embed dim
DH = 512      # d_all per core (8 heads x 64)
D = 64        # head dim
H = 8         # heads per core
NE = 8        # embed 128-tiles
ND = 4        # d_all 128-tiles (= head pairs)
NS = 16       # seq 128-tiles
NK = 16       # k 128-tiles
V1W = D + 1   # V columns per head incl. ones column
QC = 512      # query chunk width
NQC = 4       # number of query chunks


def _emit(nc, tc, ctx, d, reps=1):
    P = 128
    glob = ctx.enter_context(tc.tile_pool(name="glob", bufs=1))

    qt = glob.tile([P, 2 * S], BF16)     # QT, 2 pair lanes: [r, (p%2)*2048+s]
    kt = glob.tile([P, 2 * S], BF16)
    v1 = glob.tile([P, NS * H * V1W], BF16)  # [s%128, st*520 + h*65 + j]
    mt = glob.tile([P, NK * S], BF16)    # mask^T: [k%128, kt*2048+q]
    otn = glob.tile([P, ND * S], BF16)   # normalized out^T
    wo = glob.tile([P, ND * E], BF16)    # wO: [r, p*1024+c], d_all = 128p+r
    bq = glob.tile([P, ND], F32)
    bk = glob.tile([P, ND], F32)
    xt = glob.tile([P, NE * S], BF16)    # X^T: [r, e*2048+s], embed = 128e+r
    wq = glob.tile([P, NE * DH], BF16)   # wQ: [r, e*512+c]
    wk = glob.tile([P, NE * DH], BF16)
    wv = glob.tile([P, NE * DH], BF16)
    bvb = glob.tile([P, DH], BF16)

    # PSUM: exactly 8 banks
    ps_sc = ctx.enter_context(tc.tile_pool(name="ps_sc", bufs=2, space="PSUM"))
    ps_pj = ctx.enter_context(tc.tile_pool(name="ps_pj", bufs=2, space="PSUM"))
    ps_pv = ctx.enter_context(tc.tile_pool(name="ps_pv", bufs=1, space="PSUM"))

    # SBUF working pools
    prp = ctx.enter_context(tc.tile_pool(name="prp", bufs=3))
    nrm = ctx.enter_context(tc.tile_pool(name="nrm", bufs=1))
    ysp = ctx.enter_context(tc.tile_pool(name="ysp", bufs=1))
    drp = ctx.enter_context(tc.tile_pool(name="drp", bufs=2, space="DRAM"))

    # ones columns of V1 (V writes never touch them; constant across reps)
    nc.vector.memset(
        v1[:].rearrange("p (t h j) -> p t h j", t=NS, j=V1W)[:, :, :, D:D + 1],
        1.0,
    )

    if True:
        def dma_xt_quarter(sc4):
            for e in range(NE):
                nc.sync.dma_start(
                    xt[:, e * S + sc4 * QC: e * S + (sc4 + 1) * QC],
                    d["XT"].ap().rearrange("(e p) s -> e p s", p=P)[
                        e, :, sc4 * QC:(sc4 + 1) * QC
                    ],
                )

        def dma_head():
            # one serial DMA path: order strictly by first-use time
            nc.sync.dma_start(
                wk[:].rearrange("p (e c) -> p e c", c=DH),
                d["wK"].ap().rearrange("(e p) c -> p e c", p=P),
            )
            dma_xt_quarter(0)
            nc.sync.dma_start(
                wq[:].rearrange("p (e c) -> p e c", c=DH),
                d["wQ"].ap().rearrange("(e p) c -> p e c", p=P),
            )
            nc.sync.dma_start(bq[:], d["bQ"].ap().rearrange("(n p) -> p n", p=P))
            nc.sync.dma_start(bk[:], d["bK"].ap().rearrange("(n p) -> p n", p=P))

        # ---- emission helpers --------------------------------------------
        def emit_kq_unit(p, w_sb, out_t, b_t, sc4):
            """One projection unit: out_t[:, p*S + sc4*512 : +512].
            Evacuation runs on DVE (tensor_scalar add of the per-partition
            bias) so the ACT queue stays exp-only."""
            ps = ps_pj.tile([P, QC], F32, tag="pj")
            for e in range(NE):
                nc.tensor.matmul(
                    ps[:],
                    w_sb[:, e * DH + p * P: e * DH + (p + 1) * P],
                    xt[:, e * S + sc4 * QC: e * S + (sc4 + 1) * QC],
                    start=(e == 0), stop=(e == NE - 1),
                )
            lane = (p % 2) * S
            nc.vector.tensor_scalar_add(
                out_t[:, lane + sc4 * QC: lane + (sc4 + 1) * QC],
                ps[:], b_t[:, p:p + 1],
            )

        def emit_v_stile(k, p, wv, bvb):
            """V projection for s-tile k, head pair p only ([128, 128])."""
            ps = ps_pj.tile([P, QC], F32, tag="pj")
            for e in range(NE):
                nc.tensor.matmul(
                    ps[:, 0:2 * D],
                    xt[:, e * S + k * P: e * S + (k + 1) * P],
                    wv[:, e * DH + p * 2 * D: e * DH + (p + 1) * 2 * D],
                    start=(e == 0), stop=(e == NE - 1),
                )
            base = k * H * V1W + 2 * p * V1W
            dst = v1[:, base: base + 2 * V1W].rearrange(
                "p (h j) -> p h j", j=V1W
            )[:, :, 0:D]
            nc.vector.tensor_add(
                dst,
                ps[:, 0:2 * D].rearrange("p (h j) -> p h j", j=D),
                bvb[:, p * 2 * D:(p + 1) * 2 * D].rearrange("p (h j) -> p h j", j=D),
            )

        def attn_block(qc, p, extras):
            """Attention for query window qc (512 wide) and head pair p.
            extras: dict k -> list of closures emitted at the top of k-iter k.
            """
            qw = qc * QC
            pvh = [
                ps_pv.tile([V1W, QC], F32, tag=f"pv{h}", name=f"pv{h}")
                for h in range(2)
            ]
            pv_prev = None  # PV matmuls run one 2k-group behind so the PE
            # queue never head-of-line blocks on the exp->mask chain
            for kk in range(NK // 2):
                k0 = 2 * kk
                pr = prp.tile([P, 4 * QC], BF16, tag="pr")  # k0h0|k0h1|k1h0|k1h1
                for k in (k0, k0 + 1):
                    for fn in extras.get(k, ()):
                        fn()
                    sp = ps_sc.tile([P, 2 * QC], F32, tag="sc")
                    lane = (p % 2) * S
                    for h in range(2):
                        lo = h * D
                        nc.tensor.matmul(
                            sp[:, h * QC:(h + 1) * QC],
                            kt[lo:lo + D, lane + k * P: lane + (k + 1) * P],
                            qt[lo:lo + D, lane + qw: lane + qw + QC],
                            start=True, stop=True,
                        )
                    nc.scalar.activation(
                        pr[:, (k - k0) * 2 * QC: (k - k0 + 1) * 2 * QC],
                        sp[:], AF.Exp,
                    )
                # masked P = P * mask, in place, fused over the two k-tiles.
                # On alternating kk-groups the h1 lane goes to the otherwise
                # idle Pool engine (SBUF-only); the one-group PV deferral
                # gives it the ~2us it needs.
                mtv = mt[:].rearrange("p (k q) -> p k q", q=S)[
                    :, k0:k0 + 2, qw:qw + QC
                ]
                for h in range(2):
                    pm = pr[:].rearrange("p (a b) -> p a b", a=2)[
                        :, :, h * QC:(h + 1) * QC
                    ]
                    nc.vector.tensor_mul(pm, pm, mtv)
                if pv_prev is not None:
                    pv_prev()

                def pv_group(k0=k0, pr=pr):
                    for k in (k0, k0 + 1):
                        for h in range(2):
                            head = 2 * p + h
                            nc.tensor.matmul(
                                pvh[h][:],
                                v1[:, k * H * V1W + head * V1W:
                                      k * H * V1W + head * V1W + V1W],
                                pr[:, (k - k0) * 2 * QC + h * QC:
                                      (k - k0) * 2 * QC + (h + 1) * QC],
                                start=(k == 0), stop=(k == NK - 1),
                            )
                pv_prev = pv_group

            # The block's LAST PV group and the normalize chain are deferred
            # into the next block's early k-slots, so the next block's first
            # scores matmuls (which feed the ACT exp chain) are never stuck
            # behind them in the in-order PE queue.
            def normalize():
                # evacuate + normalize:  otn[:, p*S+qw] = pv[0:64] / rowsum
                st = nrm.tile([P, QC], BF16, tag="st", bufs=2)
                rs = nrm.tile([D + 1, 2 * QC], BF16, tag="rs", bufs=2)
                for h in range(2):
                    nc.vector.tensor_copy(st[h * D:(h + 1) * D, :], pvh[h][0:D, :])
                    with nc.allow_low_precision(reason="softmax denom recip bf16"):
                        nc.vector.reciprocal(
                            rs[D:D + 1, h * QC:(h + 1) * QC], pvh[h][D:D + 1, :]
                        )
                dsc = drp.tile([1, 2 * QC], BF16, tag="dsc")
                nc.sync.dma_start(dsc[:], rs[D:D + 1, :])
                nc.sync.dma_start(rb[0:D, :], dsc[:, 0:QC].partition_broadcast(D))
                nc.sync.dma_start(rb[D:P, :], dsc[:, QC:2 * QC].partition_broadcast(D))
                normalize.st = st

            rb = nrm.tile([P, QC], BF16, tag="rb", bufs=2)

            def finish():
                # deferred further so the broadcast DMA latency never blocks
                # the DVE queue head; runs on Pool (SBUF-only)
                nc.gpsimd.tensor_mul(
                    otn[:, p * S + qw: p * S + qw + QC], normalize.st[:], rb[:]
                )
            return [pv_prev, normalize, finish]

        def emit_y_half(qc, qi, ec, eng=None):
            """Y projection + DMA out for one [128, 512] output chunk."""
            q0 = qc * QC + qi * P
            yps = ps_pj.tile([P, QC], F32, tag="pj")
            for p in range(ND):
                nc.tensor.matmul(
                    yps[:],
                    otn[:, p * S + q0: p * S + q0 + P],
                    wo[:, p * E + ec * QC: p * E + (ec + 1) * QC],
                    start=(p == 0), stop=(p == ND - 1),
                )
            ysb = ysp.tile([P, QC], BF16, tag="ys", bufs=2)
            if eng is nc.scalar:
                nc.scalar.copy(ysb[:], yps[:])
            else:
                nc.vector.tensor_copy(ysb[:], yps[:])
            nc.sync.dma_start(
                d["Yp"].ap()[q0:q0 + P, ec * QC:(ec + 1) * QC], ysb[:]
            )

        def K_unit(p_, j_):
            return lambda: emit_kq_unit(p_, wk, kt, bk, j_)

        def Q_unit(p_, j_):
            return lambda: emit_kq_unit(p_, wq, qt, bq, j_)

        def make_extras(p, qc):
            """Block order is p-major: (p, qc) runs qc fastest.  Projection
            units for pair p+1 (K, V) are spread over pair p's four blocks;
            the Q unit for the next query chunk of this pair rides
            mid-block."""
            extras = {}

            def add(k, fn):
                extras.setdefault(k, []).append(fn)

            if p == 0 and qc == 0:
                # K units 1..3 of pair 0 just ahead of their k-tiles
                for j in (1, 2, 3):
                    add(4 * j - 2, K_unit(0, j))
            if p < ND - 1:
                if p == 0 and qc == 0:
                    pass  # keep the V/prologue block lean
                elif p == 0 and qc == 1:
                    add(4, K_unit(1, 0))
                    add(10, K_unit(1, 1))
                elif p == 0:
                    add(10, K_unit(1, qc))
                else:
                    add(10, K_unit(p + 1, qc))
                if qc == NQC - 1:
                    add(13, Q_unit(p + 1, 0))
                # V s-tiles of the next pair, 4 per block
                for i, st_ in enumerate(range(qc * 4, qc * 4 + 4)):
                    add((3, 6, 9, 14)[i],
                        (lambda s_, p_: lambda: emit_v_stile(s_, p_, wv, bvb))(
                            st_, p + 1))
            if qc < NQC - 1:
                if p == 2:
                    # pre-issue pair 3's next-chunk Q here: the p3 blocks are
                    # already loaded with the Y emissions
                    add(12, Q_unit(2, qc + 1))
                    add(14, Q_unit(3, qc + 1))
                elif p < 2:
                    add(12, Q_unit(p, qc + 1))
            return extras

        def dma_mt(k):
            nc.sync.dma_start(
                mt[:, k * S:(k + 1) * S],
                d["maskT"].ap().rearrange("(k p) q -> k p q", p=P)[k],
            )

        def dma_rest():
            # continue need-ordering: first masks, V weights, then the rest
            dma_mt(0); dma_mt(1)
            nc.sync.dma_start(
                wv[:].rearrange("p (e c) -> p e c", c=DH),
                d["wV"].ap().rearrange("(e p) c -> p e c", p=P),
            )
            nc.sync.dma_start(
                bvb[:],
                d["bV"].ap().rearrange("(a s) -> a s", a=1).partition_broadcast(P),
            )
            dma_mt(2); dma_mt(3); dma_mt(4); dma_mt(5)
            dma_xt_quarter(1)
            dma_mt(6); dma_mt(7); dma_mt(8); dma_mt(9)
            dma_xt_quarter(2)
            for k in range(10, NK):
                dma_mt(k)
            dma_xt_quarter(3)
            nc.sync.dma_start(
                wo[:].rearrange("p (n c) -> p n c", c=E),
                d["wO"].ap().rearrange("(n p) c -> p n c", p=P),
            )

        # ---- per-rep emission; pools/tiles persist so consecutive reps
        # pipeline through the same SBUF/PSUM slots --------------------------
        pending = []
        for _rep in range(reps):
            dma_head()
            dma_rest()

            # prologue: first K/Q units of pair 0
            emit_kq_unit(0, wk, kt, bk, 0)
            emit_kq_unit(0, wq, qt, bq, 0)

            for p in range(ND):
                for qc in range(NQC):
                    ex = make_extras(p, qc)
                    if p == 0 and qc == 0:
                        for k in range(NK):
                            ex.setdefault(k, []).insert(
                                0,
                                (lambda k_: lambda: emit_v_stile(k_, 0, wv, bvb))(k),
                            )
                    # consume at most 6 pending chunks per block (spread Y
                    # emissions so no block's PE budget overflows); carry the
                    # rest into the following block
                    take, carry = pending[:8], pending[8:]
                    for i, fn in enumerate(take):
                        ex.setdefault((1, 2, 3, 5, 7, 9, 11, 13)[i], []).append(fn)
                    pending = attn_block(qc, p, ex) + carry
                    if p == ND - 1:
                        # the last query chunk's Y evacuations run post-exp,
                        # when ACT is idle
                        ye = nc.scalar if qc == NQC - 1 else nc.vector
                        pending += [
                            (lambda a, b, c, e: lambda: emit_y_half(a, b, c, e))(
                                qc, qi, ec, ye)
                            for qi in range(QC // P) for ec in range(2)
                        ]
        for fn in pending:
            fn()


def build_module(reps=1):
    from contextlib import ExitStack

    nc = bacc.Bacc("TRN2", target_bir_lowering=False, debug=False)
    d = {
        "XT": nc.dram_tensor("XT", [E, S], BF16, kind="ExternalInput"),
        "maskT": nc.dram_tensor("maskT", [S, S], BF16, kind="ExternalInput"),
        "wQ": nc.dram_tensor("wQ", [E, DH], BF16, kind="ExternalInput"),
        "wK": nc.dram_tensor("wK", [E, DH], BF16, kind="ExternalInput"),
        "wV": nc.dram_tensor("wV", [E, DH], BF16, kind="ExternalInput"),
        "wO": nc.dram_tensor("wO", [DH, E], BF16, kind="ExternalInput"),
        "bQ": nc.dram_tensor("bQ", [DH], F32, kind="ExternalInput"),
        "bK": nc.dram_tensor("bK", [DH], F32, kind="ExternalInput"),
        "bV": nc.dram_tensor("bV", [DH], BF16, kind="ExternalInput"),
        "Yp": nc.dram_tensor("Yp", [S, E], BF16, kind="ExternalOutput"),
    }
    with tile.TileContext(nc) as tc:
        with ExitStack() as ctx:
            _emit(nc, tc, ctx, d, reps=reps)
    nc.compile()
    return nc


def make_in_maps(X, mask, wQ, bQ, wK, bK, wV, bV, wO, bO):
    """Per-core input dicts. Core c: batch c//2, head-half c%2."""
    in_maps = []
    for c in range(8):
        b, hh = c // 2, c % 2
        cols = slice(hh * DH, (hh + 1) * DH)
        in_maps.append({
            "XT": np.ascontiguousarray(np.asarray(X[b]).T).astype(ml_dtypes.bfloat16),
            "maskT": np.ascontiguousarray(
                np.asarray(mask[b, 0]).T
            ).astype(ml_dtypes.bfloat16),
            "wQ": (np.asarray(wQ[:, cols]) * np.float32(0.125)).astype(ml_dtypes.bfloat16),
            "wK": np.asarray(wK[:, cols]).astype(ml_dtypes.bfloat16),
            "wV": np.asarray(wV[:, cols]).astype(ml_dtypes.bfloat16),
            "wO": np.asarray(wO[cols, :]).astype(ml_dtypes.bfloat16),
            "bQ": np.ascontiguousarray(np.asarray(bQ[cols])) * np.float32(0.125),
            "bK": np.ascontiguousarray(np.asarray(bK[cols])),
            "bV": np.ascontiguousarray(np.asarray(bV[cols])).astype(ml_dtypes.bfloat16),
        })
    return in_maps


_NC = None


def kernel(X, mask, wQ, bQ, wK, bK, wV, bV, wO, bO):
    global _NC
    if _NC is None:
        _NC = build_module()
    in_maps = make_in_maps(X, mask, wQ, bQ, wK, bK, wV, bV, wO, bO)
    res = run_bass_kernel_spmd(_NC, in_maps, list(range(8)))
    B = 4
    Y = np.empty((B, S, E), dtype=np.float32)
    bO = np.asarray(bO, dtype=np.float32)
    for b in range(B):
        Y[b] = (
            np.asarray(res.results[2 * b]["Yp"], dtype=np.float32)
            + np.asarray(res.results[2 * b + 1]["Yp"], dtype=np.float32)
            + bO
        )
    return Y



# revision 28
# speedup vs baseline: 1.0907x; 1.0907x over previous
"""Multi-head self-attention Trainium2 kernel (8-core SPMD), v3.

Problem: B=4, S=2048, E=1024, 16 heads x 64 dim, int mask, softmax attention.

Sharding: core c handles batch b=c//2 and head-half hh=c%2 (8 heads).
Each core computes Yp = Attn(X[b])[:, heads(hh)] @ wO[rows(hh)]  -> [S, E]
partial product (bf16); host sums the two partials per batch and adds bO.

Engine budgets per core (all within ~10% of each other -- the kernel is a
balanced 4-engine pipeline):
  PE  ~273us: proj 197k cyc + scores 131k (the two heads of a pair run
      row-tiled on PE row groups 0-63/64-127 concurrently) + PV 262k
      (ones-column carries the softmax row-sums for free) + Y 66k.
  ACT ~266us: exp only (256 x [128,1024] PSUM->SBUF tiles).  Keeping the
      ACT queue exp-only matters: every k-tile passes through one exp, so
      ACT is the pipeline pacer and must never head-of-line block.
  DVE ~235us: mask multiplies (bf16 2x, in-place, fused over 2 k-tiles),
      K/Q projection evacuations (tensor_scalar add of the bias),
      PSUM evacuations (st/ysb), reciprocals.
  Pool ~20us: otn normalize multiplies (SBUF-only engine).

Loop structure: p-major (4 head pairs) x qc (4 query chunks of 512) x k.
Projections are software-pipelined into the attention k-loops.  Each
block's LAST PV group + normalize + finish are deferred into the next
block's early k-slots (pending/carry mechanism, max 8 chunks per block)
so the next block's scores -- which feed the exp chain -- are never stuck
behind them in the in-order PE queue.  Y(qc) chunks drain through the
same mechanism; the final query chunk's Y evacuations run on ACT
(post-exp, idle).  DMAs are issued on one queue in first-use order.

PSUM (8 banks): scores [128,1024] x2 bufs (4) + proj/Y [128,512] x2 (2) +
PV accumulators 2 x [65,512] (2).
"""

import sys

if "/opt/trn_rl_repo" not in sys.path:
    sys.path.insert(0, "/opt/trn_rl_repo")

import numpy as np
import ml_dtypes

import concourse.bass as bass
import concourse.tile as tile
from concourse import bacc, mybir
from concourse.bass_utils import run_bass_kernel_spmd

F32 = mybir.dt.float32
BF16 = mybir.dt.bfloat16
AF = mybir.ActivationFunctionType

S = 2048      # sequence length
E = 1024      # embed dim
DH = 512      # d_all per core (8 heads x 64)
D = 64        # head dim
H = 8         # heads per core
NE = 8        # embed 128-tiles
ND = 4        # d_all 128-tiles (= head pairs)
NS = 16       # seq 128-tiles
NK = 16       # k 128-tiles
V1W = D + 1   # V columns per head incl. ones column
QC = 512      # query chunk width
NQC = 4       # number of query chunks


def _emit(nc, tc, ctx, d, reps=1):
    P = 128
    glob = ctx.enter_context(tc.tile_pool(name="glob", bufs=1))

    qt = glob.tile([P, 2 * S], BF16)     # QT, 2 pair lanes: [r, (p%2)*2048+s]
    kt = glob.tile([P, 2 * S], BF16)
    v1 = glob.tile([P, NS * H * V1W], BF16)  # [s%128, st*520 + h*65 + j]
    mt = glob.tile([P, NK * S], BF16)    # mask^T: [k%128, kt*2048+q]
    otn = glob.tile([P, ND * S], BF16)   # normalized out^T
    wo = glob.tile([P, ND * E], BF16)    # wO: [r, p*1024+c], d_all = 128p+r
    bq = glob.tile([P, ND], F32)
    bk = glob.tile([P, ND], F32)
    xt = glob.tile([P, NE * S], BF16)    # X^T: [r, e*2048+s], embed = 128e+r
    wq = glob.tile([P, NE * DH], BF16)   # wQ: [r, e*512+c]
    wk = glob.tile([P, NE * DH], BF16)
    wv = glob.tile([P, NE * DH], BF16)
    bvb = glob.tile([P, DH], BF16)

    # PSUM: exactly 8 banks
    ps_sc = ctx.enter_context(tc.tile_pool(name="ps_sc", bufs=2, space="PSUM"))
    ps_pj = ctx.enter_context(tc.tile_pool(name="ps_pj", bufs=2, space="PSUM"))
    ps_pv = ctx.enter_context(tc.tile_pool(name="ps_pv", bufs=1, space="PSUM"))

    # SBUF working pools
    prp = ctx.enter_context(tc.tile_pool(name="prp", bufs=3))
    nrm = ctx.enter_context(tc.tile_pool(name="nrm", bufs=1))
    ysp = ctx.enter_context(tc.tile_pool(name="ysp", bufs=1))
    drp = ctx.enter_context(tc.tile_pool(name="drp", bufs=2, space="DRAM"))

    # ones columns of V1 (V writes never touch them; constant across reps)
    nc.vector.memset(
        v1[:].rearrange("p (t h j) -> p t h j", t=NS, j=V1W)[:, :, :, D:D + 1],
        1.0,
    )

    if True:
        def dma_xt_quarter(sc4):
            for e in range(NE):
                nc.sync.dma_start(
                    xt[:, e * S + sc4 * QC: e * S + (sc4 + 1) * QC],
                    d["XT"].ap().rearrange("(e p) s -> e p s", p=P)[
                        e, :, sc4 * QC:(sc4 + 1) * QC
                    ],
                )

        def dma_head():
            # one serial DMA path: order strictly by first-use time
            nc.sync.dma_start(
                wk[:].rearrange("p (e c) -> p e c", c=DH),
                d["wK"].ap().rearrange("(e p) c -> p e c", p=P),
            )
            dma_xt_quarter(0)
            nc.sync.dma_start(
                wq[:].rearrange("p (e c) -> p e c", c=DH),
                d["wQ"].ap().rearrange("(e p) c -> p e c", p=P),
            )
            nc.sync.dma_start(bq[:], d["bQ"].ap().rearrange("(n p) -> p n", p=P))
            nc.sync.dma_start(bk[:], d["bK"].ap().rearrange("(n p) -> p n", p=P))

        # ---- emission helpers --------------------------------------------
        def emit_kq_unit(p, w_sb, out_t, b_t, sc4):
            """One projection unit: out_t[:, p*S + sc4*512 : +512].
            Evacuation runs on DVE (tensor_scalar add of the per-partition
            bias) so the ACT queue stays exp-only."""
            ps = ps_pj.tile([P, QC], F32, tag="pj")
            for e in range(NE):
                nc.tensor.matmul(
                    ps[:],
                    w_sb[:, e * DH + p * P: e * DH + (p + 1) * P],
                    xt[:, e * S + sc4 * QC: e * S + (sc4 + 1) * QC],
                    start=(e == 0), stop=(e == NE - 1),
                )
            lane = (p % 2) * S
            nc.vector.tensor_scalar_add(
                out_t[:, lane + sc4 * QC: lane + (sc4 + 1) * QC],
                ps[:], b_t[:, p:p + 1],
            )

        def emit_v_stile(k, p, wv, bvb):
            """V projection for s-tile k, head pair p only ([128, 128])."""
            ps = ps_pj.tile([P, QC], F32, tag="pj")
            for e in range(NE):
                nc.tensor.matmul(
                    ps[:, 0:2 * D],
                    xt[:, e * S + k * P: e * S + (k + 1) * P],
                    wv[:, e * DH + p * 2 * D: e * DH + (p + 1) * 2 * D],
                    start=(e == 0), stop=(e == NE - 1),
                )
            base = k * H * V1W + 2 * p * V1W
            dst = v1[:, base: base + 2 * V1W].rearrange(
                "p (h j) -> p h j", j=V1W
            )[:, :, 0:D]
            nc.vector.tensor_add(
                dst,
                ps[:, 0:2 * D].rearrange("p (h j) -> p h j", j=D),
                bvb[:, p * 2 * D:(p + 1) * 2 * D].rearrange("p (h j) -> p h j", j=D),
            )

        def attn_block(qc, p, extras):
            """Attention for query window qc (512 wide) and head pair p.
            extras: dict k -> list of closures emitted at the top of k-iter k.
            """
            qw = qc * QC
            pvh = [
                ps_pv.tile([V1W, QC], F32, tag=f"pv{h}", name=f"pv{h}")
                for h in range(2)
            ]
            pv_prev = None  # PV matmuls run one 2k-group behind so the PE
            # queue never head-of-line blocks on the exp->mask chain
            for kk in range(NK // 2):
                k0 = 2 * kk
                pr = prp.tile([P, 4 * QC], BF16, tag="pr")  # k0h0|k0h1|k1h0|k1h1
                for k in (k0, k0 + 1):
                    for fn in extras.get(k, ()):
                        fn()
                    sp = ps_sc.tile([P, 2 * QC], F32, tag="sc")
                    lane = (p % 2) * S
                    for h in range(2):
                        lo = h * D
                        nc.tensor.matmul(
                            sp[:, h * QC:(h + 1) * QC],
                            kt[lo:lo + D, lane + k * P: lane + (k + 1) * P],
                            qt[lo:lo + D, lane + qw: lane + qw + QC],
                            start=True, stop=True,
                        )
                    nc.scalar.activation(
                        pr[:, (k - k0) * 2 * QC: (k - k0 + 1) * 2 * QC],
                        sp[:], AF.Exp,
                    )
                # masked P = P * mask, in place, fused over the two k-tiles.
                # On alternating kk-groups the h1 lane goes to the otherwise
                # idle Pool engine (SBUF-only); the one-group PV deferral
                # gives it the ~2us it needs.
                mtv = mt[:].rearrange("p (k q) -> p k q", q=S)[
                    :, k0:k0 + 2, qw:qw + QC
                ]
                for h in range(2):
                    pm = pr[:].rearrange("p (a b) -> p a b", a=2)[
                        :, :, h * QC:(h + 1) * QC
                    ]
                    nc.vector.tensor_mul(pm, pm, mtv)
                if pv_prev is not None:
                    pv_prev()

                def pv_group(k0=k0, pr=pr):
                    for k in (k0, k0 + 1):
                        for h in range(2):
                            head = 2 * p + h
                            nc.tensor.matmul(
                                pvh[h][:],
                                v1[:, k * H * V1W + head * V1W:
                                      k * H * V1W + head * V1W + V1W],
                                pr[:, (k - k0) * 2 * QC + h * QC:
                                      (k - k0) * 2 * QC + (h + 1) * QC],
                                start=(k == 0), stop=(k == NK - 1),
                            )
                pv_prev = pv_group

            # The block's LAST PV group and the normalize chain are deferred
            # into the next block's early k-slots, so the next block's first
            # scores matmuls (which feed the ACT exp chain) are never stuck
            # behind them in the in-order PE queue.
            def normalize():
                # evacuate + normalize:  otn[:, p*S+qw] = pv[0:64] / rowsum
                st = nrm.tile([P, QC], BF16, tag="st", bufs=2)
                rs = nrm.tile([D + 1, 2 * QC], BF16, tag="rs", bufs=2)
                for h in range(2):
                    nc.vector.tensor_copy(st[h * D:(h + 1) * D, :], pvh[h][0:D, :])
                    with nc.allow_low_precision(reason="softmax denom recip bf16"):
                        nc.vector.reciprocal(
                            rs[D:D + 1, h * QC:(h + 1) * QC], pvh[h][D:D + 1, :]
                        )
                dsc = drp.tile([1, 2 * QC], BF16, tag="dsc")
                nc.sync.dma_start(dsc[:], rs[D:D + 1, :])
                nc.sync.dma_start(rb[0:D, :], dsc[:, 0:QC].partition_broadcast(D))
                nc.sync.dma_start(rb[D:P, :], dsc[:, QC:2 * QC].partition_broadcast(D))
                normalize.st = st

            rb = nrm.tile([P, QC], BF16, tag="rb", bufs=2)

            def finish():
                # deferred further so the broadcast DMA latency never blocks
                # the DVE queue head; runs on Pool (SBUF-only)
                nc.gpsimd.tensor_mul(
                    otn[:, p * S + qw: p * S + qw + QC], normalize.st[:], rb[:]
                )
            return [pv_prev, normalize, finish]

        def emit_y_half(qc, qi, ec, eng=None):
            """Y projection + DMA out for one [128, 512] output chunk."""
            q0 = qc * QC + qi * P
            yps = ps_pj.tile([P, QC], F32, tag="pj")
            for p in range(ND):
                nc.tensor.matmul(
                    yps[:],
                    otn[:, p * S + q0: p * S + q0 + P],
                    wo[:, p * E + ec * QC: p * E + (ec + 1) * QC],
                    start=(p == 0), stop=(p == ND - 1),
                )
            ysb = ysp.tile([P, QC], BF16, tag="ys", bufs=2)
            if eng is nc.scalar:
                nc.scalar.copy(ysb[:], yps[:])
            else:
                nc.vector.tensor_copy(ysb[:], yps[:])
            nc.sync.dma_start(
                d["Yp"].ap()[q0:q0 + P, ec * QC:(ec + 1) * QC], ysb[:]
            )

        def K_unit(p_, j_):
            return lambda: emit_kq_unit(p_, wk, kt, bk, j_)

        def Q_unit(p_, j_):
            return lambda: emit_kq_unit(p_, wq, qt, bq, j_)

        def make_extras(p, qc):
            """Block order is p-major: (p, qc) runs qc fastest.  Projection
            units for pair p+1 (K, V) are spread over pair p's four blocks;
            the Q unit for the next query chunk of this pair rides
            mid-block."""
            extras = {}

            def add(k, fn):
                extras.setdefault(k, []).append(fn)

            if p == 0 and qc == 0:
                # K units 1..3 of pair 0 just ahead of their k-tiles
                for j in (1, 2, 3):
                    add(4 * j - 2, K_unit(0, j))
            if p < ND - 1:
                if p == 0 and qc == 0:
                    pass  # keep the V/prologue block lean
                elif p == 0 and qc == 1:
                    add(4, K_unit(1, 0))
                    add(10, K_unit(1, 1))
                elif p == 0:
                    add(10, K_unit(1, qc))
                else:
                    add(10, K_unit(p + 1, qc))
                if qc == NQC - 1:
                    add(13, Q_unit(p + 1, 0))
                # V s-tiles of the next pair, 4 per block
                for i, st_ in enumerate(range(qc * 4, qc * 4 + 4)):
                    add((3, 6, 9, 14)[i],
                        (lambda s_, p_: lambda: emit_v_stile(s_, p_, wv, bvb))(
                            st_, p + 1))
            if qc < NQC - 1:
                if p == 2:
                    # pre-issue pair 3's next-chunk Q here: the p3 blocks are
                    # already loaded with the Y emissions
                    add(12, Q_unit(2, qc + 1))
                    add(14, Q_unit(3, qc + 1))
                elif p < 2:
                    add(12, Q_unit(p, qc + 1))
            return extras

        def dma_mt(k):
            nc.sync.dma_start(
                mt[:, k * S:(k + 1) * S],
                d["maskT"].ap().rearrange("(k p) q -> k p q", p=P)[k],
            )

        def dma_rest():
            # continue need-ordering: first masks, V weights, then the rest
            dma_mt(0); dma_mt(1)
            nc.sync.dma_start(
                wv[:].rearrange("p (e c) -> p e c", c=DH),
                d["wV"].ap().rearrange("(e p) c -> p e c", p=P),
            )
            nc.sync.dma_start(
                bvb[:],
                d["bV"].ap().rearrange("(a s) -> a s", a=1).partition_broadcast(P),
            )
            dma_mt(2); dma_mt(3); dma_mt(4); dma_mt(5)
            dma_xt_quarter(1)
            dma_mt(6); dma_mt(7); dma_mt(8); dma_mt(9)
            dma_xt_quarter(2)
            for k in range(10, NK):
                dma_mt(k)
            dma_xt_quarter(3)
            nc.sync.dma_start(
                wo[:].rearrange("p (n c) -> p n c", c=E),
                d["wO"].ap().rearrange("(n p) c -> p n c", p=P),
            )

        # ---- per-rep emission; pools/tiles persist so consecutive reps
        # pipeline through the same SBUF/PSUM slots --------------------------
        pending = []
        for _rep in range(reps):
            dma_head()
            dma_rest()

            # prologue: first K/Q units of pair 0
            emit_kq_unit(0, wk, kt, bk, 0)
            emit_kq_unit(0, wq, qt, bq, 0)

            for p in range(ND):
                for qc in range(NQC):
                    ex = make_extras(p, qc)
                    if p == 0 and qc == 0:
                        for k in range(NK):
                            ex.setdefault(k, []).insert(
                                0,
                                (lambda k_: lambda: emit_v_stile(k_, 0, wv, bvb))(k),
                            )
                    # consume at most 6 pending chunks per block (spread Y
                    # emissions so no block's PE budget overflows); carry the
                    # rest into the following block
                    take, carry = pending[:8], pending[8:]
                    for i, fn in enumerate(take):
                        ex.setdefault((1, 2, 3, 5, 7, 9, 11, 13)[i], []).append(fn)
                    pending = attn_block(qc, p, ex) + carry
                    if p == ND - 1:
                        # the last query chunk's Y evacuations run post-exp,
                        # when ACT is idle
                        ye = nc.scalar if qc == NQC - 1 else nc.vector
                        pending += [
                            (lambda a, b, c, e: lambda: emit_y_half(a, b, c, e))(
                                qc, qi, ec, ye)
                            for qi in range(QC // P) for ec in range(2)
                        ]
        for fn in pending:
            fn()


def build_module(reps=1):
    from contextlib import ExitStack

    nc = bacc.Bacc("TRN2", target_bir_lowering=False, debug=False)
    d = {
        "XT": nc.dram_tensor("XT", [E, S], BF16, kind="ExternalInput"),
        "maskT": nc.dram_tensor("maskT", [S, S], BF16, kind="ExternalInput"),
        "wQ": nc.dram_tensor("wQ", [E, DH], BF16, kind="ExternalInput"),
        "wK": nc.dram_tensor("wK", [E, DH], BF16, kind="ExternalInput"),
        "wV": nc.dram_tensor("wV", [E, DH], BF16, kind="ExternalInput"),
        "wO": nc.dram_tensor("wO", [DH, E], BF16, kind="ExternalInput"),
        "bQ": nc.dram_tensor("bQ", [DH], F32, kind="ExternalInput"),
        "bK": nc.dram_tensor("bK", [DH], F32, kind="ExternalInput"),
        "bV": nc.dram_tensor("bV", [DH], BF16, kind="ExternalInput"),
        "Yp": nc.dram_tensor("Yp", [S, E], BF16, kind="ExternalOutput"),
    }
    with tile.TileContext(nc) as tc:
        with ExitStack() as ctx:
            _emit(nc, tc, ctx, d, reps=reps)
    nc.compile()
    return nc


def make_in_maps(X, mask, wQ, bQ, wK, bK, wV, bV, wO, bO):
    """Per-core input dicts. Core c: batch c//2, head-half c%2."""
    in_maps = []
    for c in range(8):
        b, hh = c // 2, c % 2
        cols = slice(hh * DH, (hh + 1) * DH)
        in_maps.append({
            "XT": np.ascontiguousarray(np.asarray(X[b]).T).astype(ml_dtypes.bfloat16),
            "maskT": np.ascontiguousarray(
                np.asarray(mask[b, 0]).T
            ).astype(ml_dtypes.bfloat16),
            "wQ": (np.asarray(wQ[:, cols]) * np.float32(0.125)).astype(ml_dtypes.bfloat16),
            "wK": np.asarray(wK[:, cols]).astype(ml_dtypes.bfloat16),
            "wV": np.asarray(wV[:, cols]).astype(ml_dtypes.bfloat16),
            "wO": np.asarray(wO[cols, :]).astype(ml_dtypes.bfloat16),
            "bQ": np.ascontiguousarray(np.asarray(bQ[cols])) * np.float32(0.125),
            "bK": np.ascontiguousarray(np.asarray(bK[cols])),
            "bV": np.ascontiguousarray(np.asarray(bV[cols])).astype(ml_dtypes.bfloat16),
        })
    return in_maps


_NC = None


def kernel(X, mask, wQ, bQ, wK, bK, wV, bV, wO, bO):
    global _NC
    if _NC is None:
        _NC = build_module()
    in_maps = make_in_maps(X, mask, wQ, bQ, wK, bK, wV, bV, wO, bO)
    res = run_bass_kernel_spmd(_NC, in_maps, list(range(8)))
    B = 4
    Y = np.empty((B, S, E), dtype=np.float32)
    bO = np.asarray(bO, dtype=np.float32)
    for b in range(B):
        Y[b] = (
            np.asarray(res.results[2 * b]["Yp"], dtype=np.float32)
            + np.asarray(res.results[2 * b + 1]["Yp"], dtype=np.float32)
            + bO
        )
    return Y



# revision 33
# speedup vs baseline: 1.2774x; 1.1712x over previous
"""Multi-head self-attention Trainium2 kernel (8-core SPMD), v3.

Problem: B=4, S=2048, E=1024, 16 heads x 64 dim, int mask, softmax attention.

Sharding: core c handles batch b=c//2 and head-half hh=c%2 (8 heads).
Each core computes Yp = Attn(X[b])[:, heads(hh)] @ wO[rows(hh)]  -> [S, E]
partial product (bf16); host sums the two partials per batch and adds bO.

Engine budgets per core (all within ~10% of each other -- the kernel is a
balanced 4-engine pipeline):
  PE  ~273us: proj 197k cyc + scores 131k (the two heads of a pair run
      row-tiled on PE row groups 0-63/64-127 concurrently) + PV 262k
      (ones-column carries the softmax row-sums for free) + Y 66k.
  ACT ~266us: exp only (256 x [128,1024] PSUM->SBUF tiles).  Keeping the
      ACT queue exp-only matters: every k-tile passes through one exp, so
      ACT is the pipeline pacer and must never head-of-line block.
  DVE ~235us: mask multiplies (bf16 2x, in-place, fused over 2 k-tiles),
      K/Q projection evacuations (tensor_scalar add of the bias),
      PSUM evacuations (st/ysb), reciprocals.  NOTE: DVE reciprocal() is
      an iterative divide, ~5.5 cyc/elem MEASURED (the cost model says 1);
      so the row-sums are broadcast FIRST (sum rows -> DRAM -> partition
      broadcast) and ONE [128,512] reciprocal runs per block instead of
      two -- halves the hidden divide cost.  reciprocal_approx_fast
      (custom DVE op) returns NaN under this runtime; do not use.
  Pool ~20us: otn normalize multiplies (SBUF-only engine).

Loop structure: p-major (4 head pairs) x qc (4 query chunks of 512) x k.
Projections are software-pipelined into the attention k-loops.  Each
block's LAST PV group + normalize + finish are deferred into the next
block's early k-slots (pending/carry mechanism, max 8 chunks per block)
so the next block's scores -- which feed the exp chain -- are never stuck
behind them in the in-order PE queue.  Y(qc) chunks drain through the
same mechanism; the final query chunk's Y evacuations run on ACT
(post-exp, idle).  DMAs are issued on one queue in first-use order.

PSUM (8 banks): scores [128,1024] x2 bufs (4) + proj/Y [128,512] x2 (2) +
PV accumulators 2 x [65,512] (2).
"""

import sys

if "/opt/trn_rl_repo" not in sys.path:
    sys.path.insert(0, "/opt/trn_rl_repo")

import numpy as np
import ml_dtypes

import concourse.bass as bass
import concourse.tile as tile
from concourse import bacc, mybir
from concourse.bass_utils import run_bass_kernel_spmd

F32 = mybir.dt.float32
BF16 = mybir.dt.bfloat16
AF = mybir.ActivationFunctionType

S = 2048      # sequence length
E = 1024      # embed dim
DH = 512      # d_all per core (8 heads x 64)
D = 64        # head dim
H = 8         # heads per core
NE = 8        # embed 128-tiles
ND = 4        # d_all 128-tiles (= head pairs)
NS = 16       # seq 128-tiles
NK = 16       # k 128-tiles
V1W = D + 1   # V columns per head incl. ones column
QC = 512      # query chunk width
NQC = 4       # number of query chunks


def _emit(nc, tc, ctx, d, reps=1):
    P = 128
    glob = ctx.enter_context(tc.tile_pool(name="glob", bufs=1))

    qt = glob.tile([P, 2 * S], BF16)     # QT, 2 pair lanes: [r, (p%2)*2048+s]
    kt = glob.tile([P, 2 * S], BF16)
    v1 = glob.tile([P, NS * H * V1W], BF16)  # [s%128, st*520 + h*65 + j]
    mt = glob.tile([P, NK * S], BF16)    # mask^T: [k%128, kt*2048+q]
    otn = glob.tile([P, ND * S], BF16)   # normalized out^T
    wo = glob.tile([P, ND * E], BF16)    # wO: [r, p*1024+c], d_all = 128p+r
    bq = glob.tile([P, ND], F32)
    bk = glob.tile([P, ND], F32)
    xt = glob.tile([P, NE * S], BF16)    # X^T: [r, e*2048+s], embed = 128e+r
    wq = glob.tile([P, NE * DH], BF16)   # wQ: [r, e*512+c]
    wk = glob.tile([P, NE * DH], BF16)
    wv = glob.tile([P, NE * DH], BF16)
    bvb = glob.tile([P, DH], BF16)

    # PSUM: exactly 8 banks
    ps_sc = ctx.enter_context(tc.tile_pool(name="ps_sc", bufs=2, space="PSUM"))
    ps_pj = ctx.enter_context(tc.tile_pool(name="ps_pj", bufs=2, space="PSUM"))
    ps_pv = ctx.enter_context(tc.tile_pool(name="ps_pv", bufs=1, space="PSUM"))

    # SBUF working pools
    prp = ctx.enter_context(tc.tile_pool(name="prp", bufs=3))
    nrm = ctx.enter_context(tc.tile_pool(name="nrm", bufs=1))
    ysp = ctx.enter_context(tc.tile_pool(name="ysp", bufs=1))
    drp = ctx.enter_context(tc.tile_pool(name="drp", bufs=2, space="DRAM"))

    # ones columns of V1 (V writes never touch them; constant across reps)
    nc.vector.memset(
        v1[:].rearrange("p (t h j) -> p t h j", t=NS, j=V1W)[:, :, :, D:D + 1],
        1.0,
    )

    if True:
        def dma_xt_quarter(sc4):
            for e in range(NE):
                nc.sync.dma_start(
                    xt[:, e * S + sc4 * QC: e * S + (sc4 + 1) * QC],
                    d["XT"].ap().rearrange("(e p) s -> e p s", p=P)[
                        e, :, sc4 * QC:(sc4 + 1) * QC
                    ],
                )

        def dma_head():
            # one serial DMA path: order strictly by first-use time
            nc.sync.dma_start(
                wk[:].rearrange("p (e c) -> p e c", c=DH),
                d["wK"].ap().rearrange("(e p) c -> p e c", p=P),
            )
            dma_xt_quarter(0)
            nc.sync.dma_start(
                wq[:].rearrange("p (e c) -> p e c", c=DH),
                d["wQ"].ap().rearrange("(e p) c -> p e c", p=P),
            )
            nc.sync.dma_start(bq[:], d["bQ"].ap().rearrange("(n p) -> p n", p=P))
            nc.sync.dma_start(bk[:], d["bK"].ap().rearrange("(n p) -> p n", p=P))

        # ---- emission helpers --------------------------------------------
        def emit_kq_unit(p, w_sb, out_t, b_t, sc4):
            """One projection unit: out_t[:, p*S + sc4*512 : +512].
            Evacuation runs on DVE (tensor_scalar add of the per-partition
            bias) so the ACT queue stays exp-only."""
            ps = ps_pj.tile([P, QC], F32, tag="pj")
            for e in range(NE):
                nc.tensor.matmul(
                    ps[:],
                    w_sb[:, e * DH + p * P: e * DH + (p + 1) * P],
                    xt[:, e * S + sc4 * QC: e * S + (sc4 + 1) * QC],
                    start=(e == 0), stop=(e == NE - 1),
                )
            lane = (p % 2) * S
            nc.vector.tensor_scalar_add(
                out_t[:, lane + sc4 * QC: lane + (sc4 + 1) * QC],
                ps[:], b_t[:, p:p + 1],
            )

        def emit_v_stile(k, p, wv, bvb):
            """V projection for s-tile k, head pair p only ([128, 128])."""
            ps = ps_pj.tile([P, QC], F32, tag="pj")
            for e in range(NE):
                nc.tensor.matmul(
                    ps[:, 0:2 * D],
                    xt[:, e * S + k * P: e * S + (k + 1) * P],
                    wv[:, e * DH + p * 2 * D: e * DH + (p + 1) * 2 * D],
                    start=(e == 0), stop=(e == NE - 1),
                )
            base = k * H * V1W + 2 * p * V1W
            dst = v1[:, base: base + 2 * V1W].rearrange(
                "p (h j) -> p h j", j=V1W
            )[:, :, 0:D]
            nc.vector.tensor_add(
                dst,
                ps[:, 0:2 * D].rearrange("p (h j) -> p h j", j=D),
                bvb[:, p * 2 * D:(p + 1) * 2 * D].rearrange("p (h j) -> p h j", j=D),
            )

        def attn_block(qc, p, extras):
            """Attention for query window qc (512 wide) and head pair p.
            extras: dict k -> list of closures emitted at the top of k-iter k.
            """
            qw = qc * QC
            pvh = [
                ps_pv.tile([V1W, QC], F32, tag=f"pv{h}", name=f"pv{h}")
                for h in range(2)
            ]
            pv_prev = None  # PV matmuls run one 2k-group behind so the PE
            # queue never head-of-line blocks on the exp->mask chain
            for kk in range(NK // 2):
                k0 = 2 * kk
                pr = prp.tile([P, 4 * QC], BF16, tag="pr")  # k0h0|k0h1|k1h0|k1h1
                for k in (k0, k0 + 1):
                    for fn in extras.get(k, ()):
                        fn()
                    sp = ps_sc.tile([P, 2 * QC], F32, tag="sc")
                    lane = (p % 2) * S
                    for h in range(2):
                        lo = h * D
                        nc.tensor.matmul(
                            sp[:, h * QC:(h + 1) * QC],
                            kt[lo:lo + D, lane + k * P: lane + (k + 1) * P],
                            qt[lo:lo + D, lane + qw: lane + qw + QC],
                            start=True, stop=True,
                        )
                    nc.scalar.activation(
                        pr[:, (k - k0) * 2 * QC: (k - k0 + 1) * 2 * QC],
                        sp[:], AF.Exp,
                    )
                # masked P = P * mask, in place, fused over the two k-tiles.
                # On alternating kk-groups the h1 lane goes to the otherwise
                # idle Pool engine (SBUF-only); the one-group PV deferral
                # gives it the ~2us it needs.
                mtv = mt[:].rearrange("p (k q) -> p k q", q=S)[
                    :, k0:k0 + 2, qw:qw + QC
                ]
                for h in range(2):
                    pm = pr[:].rearrange("p (a b) -> p a b", a=2)[
                        :, :, h * QC:(h + 1) * QC
                    ]
                    nc.vector.tensor_mul(pm, pm, mtv)
                if pv_prev is not None:
                    pv_prev()

                def pv_group(k0=k0, pr=pr):
                    for k in (k0, k0 + 1):
                        for h in range(2):
                            head = 2 * p + h
                            nc.tensor.matmul(
                                pvh[h][:],
                                v1[:, k * H * V1W + head * V1W:
                                      k * H * V1W + head * V1W + V1W],
                                pr[:, (k - k0) * 2 * QC + h * QC:
                                      (k - k0) * 2 * QC + (h + 1) * QC],
                                start=(k == 0), stop=(k == NK - 1),
                            )
                pv_prev = pv_group

            # The block's LAST PV group and the normalize chain are deferred
            # into the next block's early k-slots, so the next block's first
            # scores matmuls (which feed the ACT exp chain) are never stuck
            # behind them in the in-order PE queue.
            def normalize():
                # evacuate + normalize:  otn[:, p*S+qw] = pv[0:64] / rowsum.
                # DVE reciprocal() is an iterative divide (~5.5 cyc/elem
                # measured); reciprocal_approx_fast runs at ~copy speed and
                # its ~51 ULP error is far below the bf16 rounding below.
                st = nrm.tile([P, QC], BF16, tag="st", bufs=2)
                srow = nrm.tile([1, 2 * QC], F32, tag="srow", bufs=1)
                rbf = nrm.tile([P, QC], F32, tag="rbf", bufs=1)
                for h in range(2):
                    nc.vector.tensor_copy(st[h * D:(h + 1) * D, :], pvh[h][0:D, :])
                    nc.vector.tensor_copy(
                        srow[:, h * QC:(h + 1) * QC], pvh[h][D:D + 1, :]
                    )
                dsc = drp.tile([1, 2 * QC], F32, tag="dsc")
                nc.sync.dma_start(dsc[:], srow[:])
                nc.sync.dma_start(rbf[0:D, :], dsc[:, 0:QC].partition_broadcast(D))
                nc.sync.dma_start(rbf[D:P, :], dsc[:, QC:2 * QC].partition_broadcast(D))
                with nc.allow_low_precision(reason="softmax denom recip bf16"):
                    nc.vector.reciprocal(rb[:], rbf[:])
                normalize.st = st

            rb = nrm.tile([P, QC], BF16, tag="rb", bufs=2)

            def finish():
                # deferred further so the broadcast DMA latency never blocks
                # the DVE queue head; runs on Pool (SBUF-only)
                nc.gpsimd.tensor_mul(
                    otn[:, p * S + qw: p * S + qw + QC], normalize.st[:], rb[:]
                )
            return [pv_prev, normalize, finish]

        def emit_y_half(qc, qi, ec, eng=None):
            """Y projection + DMA out for one [128, 512] output chunk."""
            q0 = qc * QC + qi * P
            yps = ps_pj.tile([P, QC], F32, tag="pj")
            for p in range(ND):
                nc.tensor.matmul(
                    yps[:],
                    otn[:, p * S + q0: p * S + q0 + P],
                    wo[:, p * E + ec * QC: p * E + (ec + 1) * QC],
                    start=(p == 0), stop=(p == ND - 1),
                )
            ysb = ysp.tile([P, QC], BF16, tag="ys", bufs=2)
            if eng is nc.scalar:
                nc.scalar.copy(ysb[:], yps[:])
            else:
                nc.vector.tensor_copy(ysb[:], yps[:])
            nc.sync.dma_start(
                d["Yp"].ap()[q0:q0 + P, ec * QC:(ec + 1) * QC], ysb[:]
            )

        def K_unit(p_, j_):
            return lambda: emit_kq_unit(p_, wk, kt, bk, j_)

        def Q_unit(p_, j_):
            return lambda: emit_kq_unit(p_, wq, qt, bq, j_)

        def make_extras(p, qc):
            """Block order is p-major: (p, qc) runs qc fastest.  Projection
            units for pair p+1 (K, V) are spread over pair p's four blocks;
            the Q unit for the next query chunk of this pair rides
            mid-block."""
            extras = {}

            def add(k, fn):
                extras.setdefault(k, []).append(fn)

            if p == 0 and qc == 0:
                # K units 1..3 of pair 0 just ahead of their k-tiles
                for j in (1, 2, 3):
                    add(4 * j - 2, K_unit(0, j))
            if p < ND - 1:
                if p == 0 and qc == 0:
                    pass  # keep the V/prologue block lean
                elif p == 0 and qc == 1:
                    add(4, K_unit(1, 0))
                    add(10, K_unit(1, 1))
                elif p == 0:
                    add(10, K_unit(1, qc))
                else:
                    add(10, K_unit(p + 1, qc))
                if qc == NQC - 1:
                    add(13, Q_unit(p + 1, 0))
                # V s-tiles of the next pair, 4 per block
                for i, st_ in enumerate(range(qc * 4, qc * 4 + 4)):
                    add((3, 6, 9, 14)[i],
                        (lambda s_, p_: lambda: emit_v_stile(s_, p_, wv, bvb))(
                            st_, p + 1))
            if qc < NQC - 1:
                if p == 2:
                    # pre-issue pair 3's next-chunk Q here: the p3 blocks are
                    # already loaded with the Y emissions
                    add(12, Q_unit(2, qc + 1))
                    add(14, Q_unit(3, qc + 1))
                elif p < 2:
                    add(12, Q_unit(p, qc + 1))
            return extras

        def dma_mt(k):
            nc.sync.dma_start(
                mt[:, k * S:(k + 1) * S],
                d["maskT"].ap().rearrange("(k p) q -> k p q", p=P)[k],
            )

        def dma_rest():
            # continue need-ordering: first masks, V weights, then the rest
            dma_mt(0); dma_mt(1)
            nc.sync.dma_start(
                wv[:].rearrange("p (e c) -> p e c", c=DH),
                d["wV"].ap().rearrange("(e p) c -> p e c", p=P),
            )
            nc.sync.dma_start(
                bvb[:],
                d["bV"].ap().rearrange("(a s) -> a s", a=1).partition_broadcast(P),
            )
            dma_mt(2); dma_mt(3); dma_mt(4); dma_mt(5)
            dma_xt_quarter(1)
            dma_mt(6); dma_mt(7); dma_mt(8); dma_mt(9)
            dma_xt_quarter(2)
            for k in range(10, NK):
                dma_mt(k)
            dma_xt_quarter(3)
            nc.sync.dma_start(
                wo[:].rearrange("p (n c) -> p n c", c=E),
                d["wO"].ap().rearrange("(n p) c -> p n c", p=P),
            )

        # ---- per-rep emission; pools/tiles persist so consecutive reps
        # pipeline through the same SBUF/PSUM slots --------------------------
        pending = []
        for _rep in range(reps):
            dma_head()
            dma_rest()

            # prologue: first K/Q units of pair 0
            emit_kq_unit(0, wk, kt, bk, 0)
            emit_kq_unit(0, wq, qt, bq, 0)

            for p in range(ND):
                for qc in range(NQC):
                    ex = make_extras(p, qc)
                    if p == 0 and qc == 0:
                        for k in range(NK):
                            ex.setdefault(k, []).insert(
                                0,
                                (lambda k_: lambda: emit_v_stile(k_, 0, wv, bvb))(k),
                            )
                    # consume at most 6 pending chunks per block (spread Y
                    # emissions so no block's PE budget overflows); carry the
                    # rest into the following block
                    take, carry = pending[:8], pending[8:]
                    for i, fn in enumerate(take):
                        ex.setdefault((1, 2, 3, 5, 7, 9, 11, 13)[i], []).append(fn)
                    pending = attn_block(qc, p, ex) + carry
                    if p == ND - 1:
                        # the last query chunk's Y evacuations run post-exp,
                        # when ACT is idle
                        ye = nc.scalar if qc == NQC - 1 else nc.vector
                        pending += [
                            (lambda a, b, c, e: lambda: emit_y_half(a, b, c, e))(
                                qc, qi, ec, ye)
                            for qi in range(QC // P) for ec in range(2)
                        ]
        for fn in pending:
            fn()


def build_module(reps=1):
    from contextlib import ExitStack

    nc = bacc.Bacc("TRN2", target_bir_lowering=False, debug=False)
    d = {
        "XT": nc.dram_tensor("XT", [E, S], BF16, kind="ExternalInput"),
        "maskT": nc.dram_tensor("maskT", [S, S], BF16, kind="ExternalInput"),
        "wQ": nc.dram_tensor("wQ", [E, DH], BF16, kind="ExternalInput"),
        "wK": nc.dram_tensor("wK", [E, DH], BF16, kind="ExternalInput"),
        "wV": nc.dram_tensor("wV", [E, DH], BF16, kind="ExternalInput"),
        "wO": nc.dram_tensor("wO", [DH, E], BF16, kind="ExternalInput"),
        "bQ": nc.dram_tensor("bQ", [DH], F32, kind="ExternalInput"),
        "bK": nc.dram_tensor("bK", [DH], F32, kind="ExternalInput"),
        "bV": nc.dram_tensor("bV", [DH], BF16, kind="ExternalInput"),
        "Yp": nc.dram_tensor("Yp", [S, E], BF16, kind="ExternalOutput"),
    }
    with tile.TileContext(nc) as tc:
        with ExitStack() as ctx:
            _emit(nc, tc, ctx, d, reps=reps)
    nc.compile()
    return nc


def make_in_maps(X, mask, wQ, bQ, wK, bK, wV, bV, wO, bO):
    """Per-core input dicts. Core c: batch c//2, head-half c%2."""
    in_maps = []
    for c in range(8):
        b, hh = c // 2, c % 2
        cols = slice(hh * DH, (hh + 1) * DH)
        in_maps.append({
            "XT": np.ascontiguousarray(np.asarray(X[b]).T).astype(ml_dtypes.bfloat16),
            "maskT": np.ascontiguousarray(
                np.asarray(mask[b, 0]).T
            ).astype(ml_dtypes.bfloat16),
            "wQ": (np.asarray(wQ[:, cols]) * np.float32(0.125)).astype(ml_dtypes.bfloat16),
            "wK": np.asarray(wK[:, cols]).astype(ml_dtypes.bfloat16),
            "wV": np.asarray(wV[:, cols]).astype(ml_dtypes.bfloat16),
            "wO": np.asarray(wO[cols, :]).astype(ml_dtypes.bfloat16),
            "bQ": np.ascontiguousarray(np.asarray(bQ[cols])) * np.float32(0.125),
            "bK": np.ascontiguousarray(np.asarray(bK[cols])),
            "bV": np.ascontiguousarray(np.asarray(bV[cols])).astype(ml_dtypes.bfloat16),
        })
    return in_maps


_NC = None


def kernel(X, mask, wQ, bQ, wK, bK, wV, bV, wO, bO):
    global _NC
    if _NC is None:
        _NC = build_module()
    in_maps = make_in_maps(X, mask, wQ, bQ, wK, bK, wV, bV, wO, bO)
    res = run_bass_kernel_spmd(_NC, in_maps, list(range(8)))
    B = 4
    Y = np.empty((B, S, E), dtype=np.float32)
    bO = np.asarray(bO, dtype=np.float32)
    for b in range(B):
        Y[b] = (
            np.asarray(res.results[2 * b]["Yp"], dtype=np.float32)
            + np.asarray(res.results[2 * b + 1]["Yp"], dtype=np.float32)
            + bO
        )
    return Y

